# revision 6
# baseline (speedup 1.0000x reference)
"""BinaryConv (XNOR-style binary-weight 3x3 conv) on 8 Trainium2 NeuronCores.

Full-input contract: kernel(x=[32,256,56,56] f32, weight=[256,256,3,3] f32)
-> [32,256,56,56] f32.

Strategy: data-parallel over batch (4 images/core), weight replicated.
Per core, an implicit GEMM over the 9 conv taps:
  out[co, h*56+w] = a[co] * sum_{ci,kh,kw} sign(w)[co,ci,kh,kw] * x_pad[ci, h+kh, w+kw]
The 5 edge/center taps run as bf16 matmuls (sign(w) exact +-1 in bf16);
the 4 corner taps run as fp8e4m3 DoubleRow matmuls at 2x PE rate, each
contracting both 128-channel input chunks in one instruction.  For the
fp8 taps x ships as e4m3(16*x) and the stationary operand is
sign(w)/16 (both exact in e4m3), so the PE products are exact and the
only approximation is the e4m3 quantization of x on those taps
(measured rel err 1.76e-2 vs the f32 reference, under the 2e-2 gate).
The fp32 scale a[co]=mean|w[co]| is computed on device from the exact
f32 weight and applied at PSUM eviction in fp32.

Host-side marshalling is layout/dtype only: x ships zero-padded in
bf16 and (scaled) e4m3 so DMAs land directly in the padded SBUF tiles
with no on-device memset/copy work; the weight ships as original f32
[O,I,3,3] (feeds the |w| reduction) plus bf16 transposes laid out so
sign() writes straight into the packed stationary-operand buffers.
PE warmup matmuls hold the HAM clock through the DMA-bound ramp;
input DMAs are latency-ordered on the sync + pool HWDGE rings; weight
f32 rides the scalar ring ahead of the output stores; PSUM
eviction+scale runs on DVE; output stores ride the scalar ring.
"""

import ml_dtypes
import numpy as np

import concourse.mybir as mybir
import concourse.tile as tile
from concourse import bacc
from concourse.bass_utils import run_bass_kernel_spmd

F32 = mybir.dt.float32
BF16 = mybir.dt.bfloat16
FP8 = mybir.dt.float8e4

N_CORES = 8
B, C, H, W = 32, 256, 56, 56
O, KH, KW = 256, 3, 3
BP = B // N_CORES            # images per core
PH, PW = H + 2, W + 2        # padded spatial
NPAD = PH * PW               # 3364
P = 128                      # partitions
NCI = C // P                 # input-channel chunks
NCO = O // P                 # output-channel chunks
HT = 8                       # output rows per psum tile
NFREE = HT * W               # 448 <= 512 fp32 psum bank
NHT = H // HT                # 7
NTAP = KH * KW               # 9
KIN = C * NTAP               # 2304 = per-filter fan-in
TAPS8 = (0, 2, 6, 8)         # corner taps -> fp8 DoubleRow
TAPSB = (1, 3, 4, 5, 7)      # edge+center taps -> bf16
NT8 = len(TAPS8)
NTB = len(TAPSB)
WTFB = NCI * NTB * NCO * P   # 2560 = packed bf16 lhsT free size
WTF8 = NT8 * NCO * NCI * P   # 2048 = packed fp8 lhsT free size
TOPROWS = 30                 # first-image padded-row split (covers ht0..2)
NTOP = TOPROWS * PW


def _wtb_off(ci: int, co: int, tb: int) -> int:
    return ((ci * NTB + tb) * NCO + co) * P


def _wt8_off(t8: int, co: int) -> int:
    return ((t8 * NCO + co) * NCI) * P


def build(bp: int = BP):
    """Build + compile the per-core program for `bp` images per core."""
    nc = bacc.Bacc(
        "TRN2",
        target_bir_lowering=False,
        debug=False,
        enable_asserts=False,
        num_devices=N_CORES,
        enable_partition_id=False,
    )
    # zero-padded inputs, channel-major: [bp, 256, 58*58]
    xb_d = nc.dram_tensor("xb", [bp, C, NPAD], BF16, kind="ExternalInput")
    x8_d = nc.dram_tensor("x8", [bp, C, NPAD], FP8, kind="ExternalInput")
    w_d = nc.dram_tensor("w", [O, C, KH, KW], F32, kind="ExternalInput")
    # wpb[p, ((ci*NTB+tb)*NCO+co)*P+o] = bf16(w[co*P+o, ci*P+p, TAPSB[tb]])
    wpb_d = nc.dram_tensor("wpb", [P, WTFB], BF16, kind="ExternalInput")
    # wp8[p, ((t8*NCO+co)*NCI+i)*P+o] = bf16(w[co*P+o, i*P+p, TAPS8[t8]])
    wp8_d = nc.dram_tensor("wp8", [P, WTF8], BF16, kind="ExternalInput")
    out_d = nc.dram_tensor("out", [bp, O, H * W], F32, kind="ExternalOutput")

    xb = xb_d.ap().rearrange("n (i p) f -> n p i f", p=P)
    x8 = x8_d.ap().rearrange("n (i p) f -> n p i f", p=P)
    w = w_d.ap().rearrange("o i kh kw -> o (i kh kw)")
    wpb = wpb_d.ap()
    out = out_d.ap()

    with tile.TileContext(nc) as tc:
        with (
            tc.tile_pool(name="const", bufs=1) as const_pool,
            tc.tile_pool(name="wstage", bufs=2) as wstage_pool,
            tc.tile_pool(name="xpadb", bufs=2) as xpadb_pool,
            tc.tile_pool(name="xpad8", bufs=2) as xpad8_pool,
            tc.tile_pool(name="otile", bufs=8) as out_pool,
            tc.tile_pool(name="psum", bufs=7, space="PSUM") as psum_pool,
            tc.tile_pool(name="warmps", bufs=1, space="PSUM") as warmps_pool,
        ):
            # ---- PE warmup: keep HAM at speed while inputs stream in -----
            warm_l = const_pool.tile([P, P], BF16)
            warm_r = const_pool.tile([P, 512], BF16)
            nc.gpsimd.memset(warm_l[:], 0.0)
            nc.gpsimd.memset(warm_r[:], 0.0)
            zbias = const_pool.tile([P, 1], F32)
            zscr = const_pool.tile([P, 1], F32)
            nc.gpsimd.memset(zbias[:], 0.0)
            warm_ps = warmps_pool.tile([P, 512], F32)
            N_WARM = 22
            for _ in range(N_WARM):
                nc.tensor.matmul(warm_ps[:], warm_l[:], warm_r[:],
                                 start=True, stop=True)
            # preload the Sign LUT on ACT before the weights arrive
            nc.scalar.sign(zscr[:], zbias[:], bias=zbias[:])

            # packed stationary operands
            wtb = const_pool.tile([P, WTFB], BF16)     # bf16 signs
            wpsb = const_pool.tile([P, WTFB], BF16)    # bf16 sign source
            wt8 = const_pool.tile([P, WTF8], FP8)      # fp8 signs / 16
            wps8 = const_pool.tile([P, WTF8], BF16)    # fp8 sign source
            sgn8 = const_pool.tile([P, WTF8], BF16)
            a_all = const_pool.tile([P, NCO], F32)
            HALFB = WTFB // NCI

            def xpad_alloc():
                xpb_t = xpadb_pool.tile([P, NCI, NPAD], BF16, name="xpb")
                xp8_t = xpad8_pool.tile([P, NCI, NPAD], FP8, name="xp8")
                return xpb_t, xp8_t

            # ---- critical-path input DMAs --------------------------------
            # All img0 input rides the sync ring in need-order so the ramp
            # gets the full HBM bandwidth.  The pool ring (fp8 weights+x)
            # and scalar ring (f32 weights for |w|) are held back behind
            # flag tiles written mid-way through the sync stream, so they
            # don't steal bandwidth from the critical prefix.
            xpads0 = xpad_alloc()
            xpb0, xp80 = xpads0
            flag_a = const_pool.tile([P, 2], BF16)
            flag_b = const_pool.tile([P, 2], BF16)
            dum_a = const_pool.tile([P, 2], BF16)
            dum_b = const_pool.tile([P, 2], BF16)
            nc.sync.dma_start(wpsb[:, :HALFB], wpb[:, :HALFB])
            nc.sync.dma_start(xpb0[:, 0, :NTOP], xb[0, :, 0, :NTOP])
            nc.sync.dma_start(flag_a[:], xb[0, :, 0, 0:2])
            nc.sync.dma_start(xpb0[:, 0, NTOP:], xb[0, :, 0, NTOP:])
            nc.sync.dma_start(flag_b[:], xb[0, :, 0, 2:4])
            nc.sync.dma_start(wpsb[:, HALFB:], wpb[:, HALFB:])
            nc.sync.dma_start(xpb0[:, 1, :NTOP], xb[0, :, 1, :NTOP])
            nc.sync.dma_start(xpb0[:, 1, NTOP:], xb[0, :, 1, NTOP:])
            # pool ring: delayed until flag_a (x-bf16 ci0 top) has landed
            nc.gpsimd.tensor_copy(dum_a[:], flag_a[:])
            nc.gpsimd.dma_start(wps8[:], wp8_d.ap())
            nc.gpsimd.dma_start(xp80[:, :, :NTOP], x8[0, :, :, :NTOP])
            nc.gpsimd.dma_start(xp80[:, :, NTOP:], x8[0, :, :, NTOP:])
            # scalar ring: delayed until flag_b (x-bf16 ci0 bot) has landed
            wstages = [wstage_pool.tile([P, KIN], F32, name="ws")
                       for _ in range(NCO)]

            # ---- signs (ACT) straight into the packed lhsT buffers -------
            QB = HALFB // 2
            for q in range(2):
                nc.scalar.sign(wtb[:, q * QB:(q + 1) * QB],
                               wpsb[:, q * QB:(q + 1) * QB], bias=zbias[:])
            nc.scalar.copy(dum_b[:], flag_b[:])
            nc.scalar.dma_start(wstages[0][:], w[0:P, :])
            nc.scalar.dma_start(wstages[1][:], w[P:2 * P, :])
            for q in range(2, 4):
                nc.scalar.sign(wtb[:, q * QB:(q + 1) * QB],
                               wpsb[:, q * QB:(q + 1) * QB], bias=zbias[:])
            Q8 = WTF8 // 2
            for q in range(2):
                nc.scalar.sign(sgn8[:, q * Q8:(q + 1) * Q8],
                               wps8[:, q * Q8:(q + 1) * Q8], bias=zbias[:])
            # fp8 stationary = sign/16 (DVE converts bf16 -> e4m3)
            nc.vector.tensor_scalar_mul(wt8[:], sgn8[:], 1.0 / 16.0)

            # |w| means (DVE) — per co chunk, emitted inside the main loop
            def emit_reduce(c2):
                asum = wstage_pool.tile([P, 1], F32, name="asum", bufs=2)
                nc.vector.tensor_reduce(
                    asum[:], wstages[c2][:], axis=mybir.AxisListType.X,
                    op=mybir.AluOpType.add, apply_absolute_value=True,
                )
                nc.vector.tensor_scalar_mul(
                    a_all[:, c2:c2 + 1], asum[:], 1.0 / KIN
                )

            # ---- main conv loop ------------------------------------------
            xpads = xpads0
            for n in range(bp):
                xpb_t, xp8_t = xpads
                xbv = xpb_t[:].rearrange("p i (h w) -> p i h w", w=PW)
                x8v = xp8_t[:].rearrange("p i (h w) -> p i h w", w=PW)
                # prefetch next image's DMAs (sync: bf16, pool: fp8)
                if n + 1 < bp:
                    nxt = xpad_alloc()
                    nc.sync.dma_start(nxt[0][:], xb[n + 1])
                    nc.gpsimd.dma_start(nxt[1][:], x8[n + 1])

                for co in range(NCO):
                    if n == 0:
                        emit_reduce(co)

                    def emit_bf16(ps, ht, ci, start):
                        for k, t in enumerate(TAPSB):
                            kh, kw = divmod(t, KW)
                            r0 = ht * HT + kh
                            rhs = xbv[:, ci, r0:r0 + HT, kw:kw + W]
                            off = _wtb_off(ci, co, k)
                            nc.tensor.matmul(
                                ps[:], wtb[:, off:off + P], rhs,
                                start=(start and k == 0), stop=False,
                            )

                    def emit_dr(ps, ht):
                        for k, t in enumerate(TAPS8):
                            kh, kw = divmod(t, KW)
                            r0 = ht * HT + kh
                            rhs = x8v[:, :, r0:r0 + HT, kw:kw + W]
                            off = _wt8_off(k, co)
                            nc.tensor.matmul(
                                ps[:],
                                wt8[:, off:off + NCI * P].rearrange(
                                    "p (i o) -> p i o", i=NCI),
                                rhs,
                                start=False, stop=(k == NT8 - 1),
                                perf_mode=mybir.MatmulPerfMode.DoubleRow,
                            )

                    def emit_tail(ps, ht):
                        ot = out_pool.tile([P, NFREE], F32, name="ot")
                        nc.vector.tensor_scalar_mul(
                            ot[:], ps[:], a_all[:, co:co + 1]
                        )
                        nc.scalar.dma_start(
                            out[n, co * P:(co + 1) * P,
                                ht * NFREE:(ht + 1) * NFREE],
                            ot[:],
                        )

                    if n == 0 and co == 0:
                        # ramp: phase the first tile group so the PE only
                        # ever waits on the earliest DMAs — all bf16 ci0
                        # taps first, then ci1, then the fp8 corner taps.
                        held = []
                        for ht in range(NHT):
                            ps = psum_pool.tile([P, NFREE], F32, name="ps")
                            emit_bf16(ps, ht, 0, start=True)
                            held.append(ps)
                        for ht in range(NHT):
                            emit_bf16(held[ht], ht, 1, start=False)
                        for ht in range(NHT):
                            emit_dr(held[ht], ht)
                            emit_tail(held[ht], ht)
                    else:
                        for ht in range(NHT):
                            ps = psum_pool.tile([P, NFREE], F32, name="ps")
                            emit_bf16(ps, ht, 0, start=True)
                            emit_bf16(ps, ht, 1, start=False)
                            emit_dr(ps, ht)
                            emit_tail(ps, ht)
                if n + 1 < bp:
                    xpads = nxt

    nc.compile()
    return nc


_NC_CACHE: dict[int, object] = {}


def _get_nc(bp: int = BP):
    if bp not in _NC_CACHE:
        _NC_CACHE[bp] = build(bp)
    return _NC_CACHE[bp]


def make_in_maps(x: np.ndarray, weight: np.ndarray, n_cores: int = N_CORES,
                 bp: int = BP):
    x = np.ascontiguousarray(x, dtype=np.float32)
    weight = np.ascontiguousarray(weight, dtype=np.float32)

    xpad = np.zeros((B, C, PH, PW), dtype=np.float32)
    xpad[:, :, 1:1 + H, 1:1 + W] = x
    xpb = np.ascontiguousarray(xpad.reshape(B, C, NPAD)).astype(
        ml_dtypes.bfloat16)
    xp8 = (16.0 * xpad.reshape(B, C, NPAD)).astype(ml_dtypes.float8_e4m3fn)

    wb = weight.astype(ml_dtypes.bfloat16)          # sign-exact cast
    wtap = wb.reshape(O, C, NTAP)
    selb = wtap[:, :, list(TAPSB)]                   # [O, C, NTB]
    arrb = selb.reshape(NCO, P, NCI, P, NTB)         # [co, o, ci, p, tb]
    wpb = np.ascontiguousarray(
        arrb.transpose(3, 2, 4, 0, 1).reshape(P, WTFB))  # [p, (ci tb co o)]
    sel8 = wtap[:, :, list(TAPS8)]                   # [O, C, NT8]
    arr = sel8.reshape(NCO, P, NCI, P, NT8)          # [co, o, i, p, t8]
    wp8 = np.ascontiguousarray(
        arr.transpose(3, 4, 0, 2, 1).reshape(P, WTF8))  # [p, (t8 co i o)]

    return [
        {"xb": xpb[i * bp:(i + 1) * bp], "x8": xp8[i * bp:(i + 1) * bp],
         "w": weight, "wpb": wpb, "wp8": wp8}
        for i in range(n_cores)
    ]


def kernel(x: np.ndarray, weight: np.ndarray) -> np.ndarray:
    nc = _get_nc(BP)
    in_maps = make_in_maps(x, weight)
    res = run_bass_kernel_spmd(nc, in_maps, core_ids=list(range(N_CORES)))
    out = np.empty((B, O, H, W), dtype=np.float32)
    for i in range(N_CORES):
        out[i * BP:(i + 1) * BP] = res.results[i]["out"].reshape(BP, O, H, W)
    return out


# revision 7
# speedup vs baseline: 1.2218x; 1.2218x over previous
"""BinaryConv (XNOR-style binary-weight 3x3 conv) on 8 Trainium2 NeuronCores.

Full-input contract: kernel(x=[32,256,56,56] f32, weight=[256,256,3,3] f32)
-> [32,256,56,56] f32.

Strategy: data-parallel over batch (4 images/core), weight replicated.
Per core, an implicit GEMM over the 9 conv taps:
  out[co, h*56+w] = a[co] * sum_{ci,kh,kw} sign(w)[co,ci,kh,kw] * x_pad[ci, h+kh, w+kw]
The 5 edge/center taps run as bf16 matmuls (sign(w) exact +-1 in bf16);
the 4 corner taps run as fp8e4m3 DoubleRow matmuls at 2x PE rate, each
contracting both 128-channel input chunks in one instruction.  For the
fp8 taps x ships as e4m3(16*x) and the stationary operand is
sign(w)/16 (both exact in e4m3), so the PE products are exact and the
only approximation is the e4m3 quantization of x on those taps
(measured rel err 1.76e-2 vs the f32 reference, under the 2e-2 gate).
The fp32 scale a[co]=mean|w[co]| is computed on device from the exact
f32 weight and applied at PSUM eviction in fp32.

Host-side marshalling is layout/dtype only: x ships zero-padded in
bf16 and (scaled) e4m3 so DMAs land directly in the padded SBUF tiles
with no on-device memset/copy work; the weight ships as original f32
[O,I,3,3] (feeds the |w| reduction) plus bf16 transposes laid out so
sign() writes straight into the packed stationary-operand buffers.
PE warmup matmuls hold the HAM clock through the DMA-bound ramp;
input DMAs are latency-ordered on the sync + pool HWDGE rings; weight
f32 rides the scalar ring ahead of the output stores; PSUM
eviction+scale runs on DVE; output stores ride the scalar ring.
"""

import ml_dtypes
import numpy as np

import concourse.mybir as mybir
import concourse.tile as tile
from concourse import bacc
from concourse.bass_utils import run_bass_kernel_spmd

F32 = mybir.dt.float32
BF16 = mybir.dt.bfloat16
FP8 = mybir.dt.float8e4

N_CORES = 8
B, C, H, W = 32, 256, 56, 56
O, KH, KW = 256, 3, 3
BP = B // N_CORES            # images per core
PH, PW = H + 2, W + 2        # padded spatial
NPAD = PH * PW               # 3364
P = 128                      # partitions
NCI = C // P                 # input-channel chunks
NCO = O // P                 # output-channel chunks
HT = 8                       # output rows per psum tile
NFREE = HT * W               # 448 <= 512 fp32 psum bank
NHT = H // HT                # 7
NTAP = KH * KW               # 9
KIN = C * NTAP               # 2304 = per-filter fan-in
TAPS8 = (0, 2, 6, 8)         # corner taps -> fp8 DoubleRow
TAPSB = (1, 3, 4, 5, 7)      # edge+center taps -> bf16
NT8 = len(TAPS8)
NTB = len(TAPSB)
WTFB = NCI * NTB * NCO * P   # 2560 = packed bf16 lhsT free size
WTF8 = NT8 * NCO * NCI * P   # 2048 = packed fp8 lhsT free size
TOPROWS = 30                 # first-image padded-row split (covers ht0..2)
NTOP = TOPROWS * PW


def _wtb_off(ci: int, co: int, tb: int) -> int:
    return ((ci * NTB + tb) * NCO + co) * P


def _wt8_off(t8: int, co: int) -> int:
    return ((t8 * NCO + co) * NCI) * P


def build(bp: int = BP):
    """Build + compile the per-core program for `bp` images per core."""
    nc = bacc.Bacc(
        "TRN2",
        target_bir_lowering=False,
        debug=False,
        enable_asserts=False,
        num_devices=N_CORES,
        enable_partition_id=False,
    )
    # zero-padded inputs, channel-major: [bp, 256, 58*58]
    xb_d = nc.dram_tensor("xb", [bp, C, NPAD], BF16, kind="ExternalInput")
    x8_d = nc.dram_tensor("x8", [bp, C, NPAD], FP8, kind="ExternalInput")
    w_d = nc.dram_tensor("w", [O, C, KH, KW], F32, kind="ExternalInput")
    # wpb[p, ((ci*NTB+tb)*NCO+co)*P+o] = bf16(w[co*P+o, ci*P+p, TAPSB[tb]])
    wpb_d = nc.dram_tensor("wpb", [P, WTFB], BF16, kind="ExternalInput")
    # wp8[p, ((t8*NCO+co)*NCI+i)*P+o] = bf16(w[co*P+o, i*P+p, TAPS8[t8]])
    wp8_d = nc.dram_tensor("wp8", [P, WTF8], BF16, kind="ExternalInput")
    out_d = nc.dram_tensor("out", [bp, O, H * W], F32, kind="ExternalOutput")

    xb = xb_d.ap().rearrange("n (i p) f -> n p i f", p=P)
    x8 = x8_d.ap().rearrange("n (i p) f -> n p i f", p=P)
    w = w_d.ap().rearrange("o i kh kw -> o (i kh kw)")
    wpb = wpb_d.ap()
    out = out_d.ap()

    with tile.TileContext(nc) as tc:
        with (
            tc.tile_pool(name="const", bufs=1) as const_pool,
            tc.tile_pool(name="wstage", bufs=2) as wstage_pool,
            tc.tile_pool(name="xpadb", bufs=2) as xpadb_pool,
            tc.tile_pool(name="xpad8", bufs=2) as xpad8_pool,
            tc.tile_pool(name="otile", bufs=8) as out_pool,
            tc.tile_pool(name="psum", bufs=7, space="PSUM") as psum_pool,
            tc.tile_pool(name="warmps", bufs=1, space="PSUM") as warmps_pool,
        ):
            # ---- PE warmup: keep HAM at speed while inputs stream in -----
            warm_l = const_pool.tile([P, P], BF16)
            warm_r = const_pool.tile([P, 512], BF16)
            nc.gpsimd.memset(warm_l[:], 0.0)
            nc.gpsimd.memset(warm_r[:], 0.0)
            zbias = const_pool.tile([P, 1], F32)
            zscr = const_pool.tile([P, 1], F32)
            nc.gpsimd.memset(zbias[:], 0.0)
            warm_ps = warmps_pool.tile([P, 512], F32)
            N_WARM = 22
            for _ in range(N_WARM):
                nc.tensor.matmul(warm_ps[:], warm_l[:], warm_r[:],
                                 start=True, stop=True)
            # preload the Sign LUT on ACT before the weights arrive
            nc.scalar.sign(zscr[:], zbias[:], bias=zbias[:])

            # packed stationary operands
            wtb = const_pool.tile([P, WTFB], BF16)     # bf16 signs
            wpsb = const_pool.tile([P, WTFB], BF16)    # bf16 sign source
            wt8 = const_pool.tile([P, WTF8], FP8)      # fp8 signs / 16
            wps8 = const_pool.tile([P, WTF8], BF16)    # fp8 sign source
            sgn8 = const_pool.tile([P, WTF8], BF16)
            a_all = const_pool.tile([P, NCO], F32)
            HALFB = WTFB // NCI

            def xpad_alloc():
                xpb_t = xpadb_pool.tile([P, NCI, NPAD], BF16, name="xpb")
                xp8_t = xpad8_pool.tile([P, NCI, NPAD], FP8, name="xp8")
                return xpb_t, xp8_t

            # ---- critical-path input DMAs --------------------------------
            # sync ring (FIFO): bf16 weights ci0 then x-bf16 img0 in
            # need-order; pool ring: fp8 weights + fp8 x img0; scalar ring:
            # bf16 weights ci1 then exact f32 weights for |w|, ahead of the
            # output stores.
            xpads0 = xpad_alloc()
            xpb0, xp80 = xpads0
            nc.sync.dma_start(wpsb[:, :HALFB], wpb[:, :HALFB])
            nc.sync.dma_start(xpb0[:, 0, :NTOP], xb[0, :, 0, :NTOP])
            nc.sync.dma_start(xpb0[:, 0, NTOP:], xb[0, :, 0, NTOP:])
            nc.sync.dma_start(xpb0[:, 1, :NTOP], xb[0, :, 1, :NTOP])
            nc.sync.dma_start(xpb0[:, 1, NTOP:], xb[0, :, 1, NTOP:])
            nc.gpsimd.dma_start(wps8[:], wp8_d.ap())
            nc.gpsimd.dma_start(xp80[:, :, :NTOP], x8[0, :, :, :NTOP])
            nc.gpsimd.dma_start(xp80[:, :, NTOP:], x8[0, :, :, NTOP:])
            nc.scalar.dma_start(wpsb[:, HALFB:], wpb[:, HALFB:])
            wstages = [wstage_pool.tile([P, KIN], F32, name="ws")
                       for _ in range(NCO)]
            nc.scalar.dma_start(wstages[0][:], w[0:P, :])
            nc.scalar.dma_start(wstages[1][:], w[P:2 * P, :])

            # ---- signs (ACT) straight into the packed lhsT buffers -------
            QB = HALFB // 2
            for q in range(4):
                nc.scalar.sign(wtb[:, q * QB:(q + 1) * QB],
                               wpsb[:, q * QB:(q + 1) * QB], bias=zbias[:])
            Q8 = WTF8 // 2
            for q in range(2):
                nc.scalar.sign(sgn8[:, q * Q8:(q + 1) * Q8],
                               wps8[:, q * Q8:(q + 1) * Q8], bias=zbias[:])
            # fp8 stationary = sign/16 (DVE converts bf16 -> e4m3)
            nc.vector.tensor_scalar_mul(wt8[:], sgn8[:], 1.0 / 16.0)

            # |w| means (DVE) — per co chunk, emitted inside the main loop
            def emit_reduce(c2):
                asum = wstage_pool.tile([P, 1], F32, name="asum", bufs=2)
                nc.vector.tensor_reduce(
                    asum[:], wstages[c2][:], axis=mybir.AxisListType.X,
                    op=mybir.AluOpType.add, apply_absolute_value=True,
                )
                nc.vector.tensor_scalar_mul(
                    a_all[:, c2:c2 + 1], asum[:], 1.0 / KIN
                )

            # ---- main conv loop ------------------------------------------
            xpads = xpads0
            for n in range(bp):
                xpb_t, xp8_t = xpads
                xbv = xpb_t[:].rearrange("p i (h w) -> p i h w", w=PW)
                x8v = xp8_t[:].rearrange("p i (h w) -> p i h w", w=PW)
                # prefetch next image's DMAs (sync: bf16, pool: fp8)
                if n + 1 < bp:
                    nxt = xpad_alloc()
                    nc.sync.dma_start(nxt[0][:], xb[n + 1])
                    nc.gpsimd.dma_start(nxt[1][:], x8[n + 1])

                for co in range(NCO):
                    if n == 0:
                        emit_reduce(co)

                    def emit_bf16(ps, ht, ci, start):
                        for k, t in enumerate(TAPSB):
                            kh, kw = divmod(t, KW)
                            r0 = ht * HT + kh
                            rhs = xbv[:, ci, r0:r0 + HT, kw:kw + W]
                            off = _wtb_off(ci, co, k)
                            nc.tensor.matmul(
                                ps[:], wtb[:, off:off + P], rhs,
                                start=(start and k == 0), stop=False,
                            )

                    def emit_dr(ps, ht):
                        for k, t in enumerate(TAPS8):
                            kh, kw = divmod(t, KW)
                            r0 = ht * HT + kh
                            rhs = x8v[:, :, r0:r0 + HT, kw:kw + W]
                            off = _wt8_off(k, co)
                            nc.tensor.matmul(
                                ps[:],
                                wt8[:, off:off + NCI * P].rearrange(
                                    "p (i o) -> p i o", i=NCI),
                                rhs,
                                start=False, stop=(k == NT8 - 1),
                                perf_mode=mybir.MatmulPerfMode.DoubleRow,
                            )

                    def emit_tail(ps, ht):
                        ot = out_pool.tile([P, NFREE], F32, name="ot")
                        nc.vector.tensor_scalar_mul(
                            ot[:], ps[:], a_all[:, co:co + 1]
                        )
                        nc.scalar.dma_start(
                            out[n, co * P:(co + 1) * P,
                                ht * NFREE:(ht + 1) * NFREE],
                            ot[:],
                        )

                    if n == 0 and co == 0:
                        # ramp: phase the first tile group so the PE only
                        # ever waits on the earliest DMAs — all bf16 ci0
                        # taps first, then ci1, then the fp8 corner taps.
                        held = []
                        for ht in range(NHT):
                            ps = psum_pool.tile([P, NFREE], F32, name="ps")
                            emit_bf16(ps, ht, 0, start=True)
                            held.append(ps)
                        for ht in range(NHT):
                            emit_bf16(held[ht], ht, 1, start=False)
                        for ht in range(NHT):
                            emit_dr(held[ht], ht)
                            emit_tail(held[ht], ht)
                    else:
                        for ht in range(NHT):
                            ps = psum_pool.tile([P, NFREE], F32, name="ps")
                            emit_bf16(ps, ht, 0, start=True)
                            emit_bf16(ps, ht, 1, start=False)
                            emit_dr(ps, ht)
                            emit_tail(ps, ht)
                if n + 1 < bp:
                    xpads = nxt

    nc.compile()
    return nc


_NC_CACHE: dict[int, object] = {}


def _get_nc(bp: int = BP):
    if bp not in _NC_CACHE:
        _NC_CACHE[bp] = build(bp)
    return _NC_CACHE[bp]


def make_in_maps(x: np.ndarray, weight: np.ndarray, n_cores: int = N_CORES,
                 bp: int = BP):
    x = np.ascontiguousarray(x, dtype=np.float32)
    weight = np.ascontiguousarray(weight, dtype=np.float32)

    xpad = np.zeros((B, C, PH, PW), dtype=np.float32)
    xpad[:, :, 1:1 + H, 1:1 + W] = x
    xpb = np.ascontiguousarray(xpad.reshape(B, C, NPAD)).astype(
        ml_dtypes.bfloat16)
    xp8 = (16.0 * xpad.reshape(B, C, NPAD)).astype(ml_dtypes.float8_e4m3fn)

    wb = weight.astype(ml_dtypes.bfloat16)          # sign-exact cast
    wtap = wb.reshape(O, C, NTAP)
    selb = wtap[:, :, list(TAPSB)]                   # [O, C, NTB]
    arrb = selb.reshape(NCO, P, NCI, P, NTB)         # [co, o, ci, p, tb]
    wpb = np.ascontiguousarray(
        arrb.transpose(3, 2, 4, 0, 1).reshape(P, WTFB))  # [p, (ci tb co o)]
    sel8 = wtap[:, :, list(TAPS8)]                   # [O, C, NT8]
    arr = sel8.reshape(NCO, P, NCI, P, NT8)          # [co, o, i, p, t8]
    wp8 = np.ascontiguousarray(
        arr.transpose(3, 4, 0, 2, 1).reshape(P, WTF8))  # [p, (t8 co i o)]

    return [
        {"xb": xpb[i * bp:(i + 1) * bp], "x8": xp8[i * bp:(i + 1) * bp],
         "w": weight, "wpb": wpb, "wp8": wp8}
        for i in range(n_cores)
    ]


def kernel(x: np.ndarray, weight: np.ndarray) -> np.ndarray:
    nc = _get_nc(BP)
    in_maps = make_in_maps(x, weight)
    res = run_bass_kernel_spmd(nc, in_maps, core_ids=list(range(N_CORES)))
    out = np.empty((B, O, H, W), dtype=np.float32)
    for i in range(N_CORES):
        out[i * BP:(i + 1) * BP] = res.results[i]["out"].reshape(BP, O, H, W)
    return out
